# revision 3
# baseline (speedup 1.0000x reference)
"""Trainium2 Bass kernel for EuclideanSimilarity (retrieval_knn), v2.

Per batch b (B=8, L=4096, D=128), one NeuronCore per batch element:
    projected = x @ W.T + b                      [L, D]
    q = avgpool2(x) @ W.T + b                    [L/2, D]
    power = ||q_i||^2 + ||k_j||^2 - 2 q_i.k_j    [L/2, L]
    sim = exp(-sqrt(max(power, 0)))
    k = sim @ projected                          [L/2, D]
    returns (q, k, v=k)

Key trick: the activation-table root is patched so AF.Sqrt evaluates
g(x) = exp(-sqrt(max(x, 0))) directly (cubic piecewise-poly table with
max rel err ~8e-4 over the operational power range [8, 256]).  The main
loop is then GEMM2 -> fused affine (psum + ksq_col + qsq_row, split
between DVE and Pool) -> ONE activation pass -> GEMM3, j-tile-major
with full-query strips.
"""

import os
import sys
import tempfile

for _p in ("/opt/trn_rl_repo", "/root/.axon_site/_ro/trn_rl_repo"):
    if os.path.isdir(_p) and _p not in sys.path:
        sys.path.insert(0, _p)

import numpy as np

# ---------------------------------------------------------------------------
# custom activation tables: AF.Sqrt := exp(-sqrt(max(x, 0)))
# ---------------------------------------------------------------------------
import json
import shutil
import struct


def _act_find_dir():
    from neuronxcc.driver.Job import Job
    from neuronxcc.driver.jobs.support.FindActInfo import findActInfoFile

    return os.path.dirname(findActInfoFile(Job.getPackageDir(), "gen3"))


def _act_g(x):
    return np.exp(-np.sqrt(np.maximum(x, 0.0)))


def _act_fit_cubic(lo, hi, xc):
    t = np.linspace(lo, hi, 33, dtype=np.float64) - xc
    y = _act_g(t + xc)
    w = 1.0 / np.maximum(y, 1e-300)
    A = np.stack([np.ones_like(t), t, t * t, t * t * t], axis=1)
    c, *_ = np.linalg.lstsq(A * w[:, None], y * w, rcond=None)
    return c


def _act_patch_set(dst, setname, fj):
    bkt_path = os.path.join(dst, f"{setname}_bkt.bin")
    blob = bytearray(open(bkt_path, "rb").read())
    n = len(blob) // 32
    recs = np.frombuffer(bytes(blob), dtype=np.uint32).reshape(n, 8)
    lut = {tuple(recs[i, :5]): i for i in range(n)}

    def reckey(sec):
        return tuple(int(sec[nm]["int"]) for nm in ("d0", "d1", "d2", "d3", "x"))

    def write_rec(i, d0, d1, d2, d3, x):
        vals = []
        for v in (d0, d1, d2, d3):
            v = np.float32(v)
            vals.append(float(v) if np.isfinite(v) else 0.0)
        blob[i * 32:i * 32 + 20] = struct.pack("<fffff", *vals, np.float32(x))

    for e in fj["pos_exponents"]:
        ee = e["exponent"]
        nsec = e["num_sections"]
        width = 2.0 ** ee / nsec
        for sec in e["exponent_sections"]:
            i = lut[reckey(sec)]
            s = sec["section_id"]
            lo = 2.0 ** ee + s * width
            xc = float(sec["x"]["float"])
            if -30 <= ee <= 12:
                c = _act_fit_cubic(lo, lo + width, xc)
                write_rec(i, c[0], c[1], c[2], c[3], xc)
            else:
                write_rec(i, _act_g(xc), 0.0, 0.0, 0.0, xc)
    sp = fj["saturation_points"]
    for nm, vals in (
        ("sat_point_pos_low", (1.0, 0.0, 0.0, 0.0, 0.0)),
        ("sat_point_pos_high", (0.0, 0.0, 0.0, 0.0, 0.0)),
        ("sat_point_neg_low", (1.0, 0.0, 0.0, 0.0, 0.0)),
        ("sat_point_neg_high", (1.0, 0.0, 0.0, 0.0, 0.0)),
    ):
        i = lut.get(reckey(sp[nm]))
        if i is not None:
            write_rec(i, *vals)
    open(bkt_path, "wb").write(bytes(blob))

    prof_path = os.path.join(dst, f"{setname}.json")
    prof = json.load(open(prof_path))
    for f in prof["profile_meta_data"]:
        if f["func_name"].startswith("sqrt"):
            f["fzero_result"] = 0x3F800000
            f["fpinf_result"] = 0
            f["fninf_result"] = 0x3F800000
    json.dump(prof, open(prof_path, "w"))


_ACT_ROOT = None


def ensure_custom_act_root():
    """Build the patched act-table dir once and point the compiler at it."""
    global _ACT_ROOT
    if _ACT_ROOT is not None:
        return _ACT_ROOT
    src = _act_find_dir()
    dst = os.path.join(tempfile.gettempdir(), "act_expnegsqrt_v1")
    marker = os.path.join(dst, ".done")
    if not os.path.exists(marker):
        if os.path.isdir(dst):
            shutil.rmtree(dst)
        os.makedirs(dst)
        for fn in os.listdir(src):
            shutil.copy(os.path.join(src, fn), os.path.join(dst, fn))
            os.chmod(os.path.join(dst, fn), 0o644)
        fj = json.load(open(os.path.join(
            os.path.dirname(src), "pwp_jsons", "sqrt_65536p.json")))
        for setname in ("sqrt_and_friends", "sqrt_and_others"):
            _act_patch_set(dst, setname, fj)
        open(marker, "w").write("ok")
    _ACT_ROOT = os.path.join(dst, "act_info.json")
    os.environ["BASS_ACT_ROOT_JSON_PATH"] = _ACT_ROOT
    return _ACT_ROOT


ensure_custom_act_root()

import concourse.bass as bass  # noqa: E402
import concourse.mybir as mybir  # noqa: E402
from concourse import bacc  # noqa: E402
from concourse.bass_utils import run_bass_kernel_spmd  # noqa: E402
from concourse.tile import TileContext  # noqa: E402

B, L, D = 8, 4096, 128
LQ = L // 2          # 2048 pooled queries
P = 128
NJT = L // P         # 32 j-tiles
NS = 512             # GEMM width unit (one PSUM bank)
NA = 1024            # affine slice width (2 PSUM banks)
NSLICE = LQ // NA    # 2 affine slices per j-tile
SPAN = 4             # affine slices per ACT op (4*1024 = 4096)
NSPAN = (NJT * NSLICE) // SPAN   # 16 spans
RING = 16 * NA                   # 16384 ring (4 spans)
F32 = mybir.dt.float32
F32R = mybir.dt.float32r

AF = mybir.ActivationFunctionType
ALU = mybir.AluOpType

# fraction of affine slices on DVE (rest on Pool/gpsimd)
DVE_MOD = int(os.environ.get("KDVE_MOD", "2"))   # s % DVE_MOD == 0 -> gpsimd


def build_nc(repeat=1, mode=None):
    nc = bacc.Bacc("TRN2", target_bir_lowering=False)

    xT = nc.declare_dram_parameter("xT", [P, L], F32R, isOutput=False)
    WT = nc.declare_dram_parameter("WT", [P, D], F32R, isOutput=False)
    WhT = nc.declare_dram_parameter("WhT", [P, D], F32R, isOutput=False)
    bcol_h = nc.declare_dram_parameter("bcol_h", [P, 1], F32, isOutput=False)
    b_bcast4_in = nc.declare_dram_parameter("b_bcast4", [P, 512], F32, isOutput=False)
    ones_in = nc.declare_dram_parameter("ones_mat", [P, P], F32R, isOutput=False)
    e0_in = nc.declare_dram_parameter("e0_mat", [P, P], F32R, isOutput=False)

    qT_out = nc.declare_dram_parameter("qT", [P, LQ], F32, isOutput=True)
    kT_out = nc.declare_dram_parameter("kT", [P, LQ], F32, isOutput=True)

    with TileContext(nc) as tc:
      for _rep in range(repeat):
        with (
            tc.tile_pool(name="consts", bufs=1) as consts,
            tc.tile_pool(name="big", bufs=1) as big,
            tc.tile_pool(name="work", bufs=4) as work,
        ):
            WT_sb = consts.tile([P, D], F32R)
            WhT_sb = consts.tile([P, D], F32R)
            bh_sb = consts.tile([P, 1], F32)
            b_bcast4 = consts.tile([P, 512], F32)
            ones_sb = consts.tile([P, P], F32R)
            e0_sb = consts.tile([P, P], F32R)
            nc.sync.dma_start(out=e0_sb[:], in_=e0_in[:])
            nc.sync.dma_start(out=WT_sb[:], in_=WT[:])
            nc.sync.dma_start(out=WhT_sb[:], in_=WhT[:])
            nc.sync.dma_start(out=bh_sb[:], in_=bcol_h[:])
            nc.sync.dma_start(out=b_bcast4[:], in_=b_bcast4_in[:])
            nc.sync.dma_start(out=ones_sb[:], in_=ones_in[:])

            projTm2 = big.tile([P, L], F32R)   # GEMM2 stationary (-2 proj)^T
            projnat = big.tile([P, L], F32R)   # GEMM3 stationary proj tiles
            qT_mm = big.tile([P, LQ], F32R, tag="qT_mm", name="qT_mm")
            sqscr = big.tile([P, D], F32, name="sqscr")
            qsq_bcast = big.tile([P, LQ], F32)
            ksq = consts.tile([P, NJT], F32)

            # ---- phase 1: projections, qT, ksq, qsq ----
            with (
                tc.tile_pool(name="phase1", bufs=1) as ph1,
                tc.tile_pool(name="ps1", bufs=4, space="PSUM") as ps1,
            ):
                xT_sb = ph1.tile([P, L], F32R)
                qT_sb = ph1.tile([P, LQ], F32, tag="qT_sb", name="qT_sb")
                for c in range(L // 512):
                    nc.sync.dma_start(
                        out=xT_sb[:, c * 512:(c + 1) * 512],
                        in_=xT[:, c * 512:(c + 1) * 512])

                # per chunk c: GEMM1 (0.5W) -> ACT bias-epilogue -> Pool
                # pair-sum for qT; projnat GEMM (4 tiles/bank) -> DVE
                # epilogue -> Pool square+accum for ksq.
                for c in range(L // 512):
                    ps = ps1.tile([P, 512], F32, tag="ps1")
                    nc.tensor.matmul(
                        ps, WhT_sb[:], xT_sb[:, c * 512:(c + 1) * 512],
                        start=True, stop=True,
                    )
                    dst = projTm2[:, c * 512:(c + 1) * 512]
                    nc.vector.tensor_scalar_add(dst, ps, bh_sb[:, 0:1])
                    sp = dst.bitcast(F32).rearrange(
                        "p (i two) -> p i two", two=2)
                    nc.vector.tensor_add(
                        qT_sb[:, c * 256:(c + 1) * 256], sp[:, :, 0],
                        sp[:, :, 1])

                nc.sync.dma_start(out=qT_out[:], in_=qT_sb[:])
                nc.gpsimd.tensor_scalar_mul(qT_mm[:], qT_sb[:], -4.0)

                # qsq row -> broadcast to all partitions via ones matmul
                sq_qT = ph1.tile([P, LQ], F32R)
                nc.vector.tensor_mul(sq_qT[:], qT_sb[:], qT_sb[:])
                for c in range(LQ // 512):
                    ps = ps1.tile([P, 512], F32, tag="ps1")
                    nc.tensor.matmul(
                        ps, ones_sb[:], sq_qT[:, c * 512:(c + 1) * 512],
                        start=True, stop=True,
                    )
                    nc.scalar.copy(qsq_bcast[:, c * 512:(c + 1) * 512], ps)

                # projnat + early ksq (tiles 0..7); later tiles' squares are
                # interleaved into the main loop where ACT has slack
                for c in range(L // 512):
                    psn = ps1.tile([P, 512], F32, tag="psn")
                    for k in range(4):
                        t = c * 4 + k
                        nc.tensor.matmul(
                            psn[:, k * D:(k + 1) * D],
                            xT_sb[:, t * P:(t + 1) * P], WT_sb[:],
                            start=True, stop=True,
                        )
                    dstn = projnat[:, c * 512:(c + 1) * 512]
                    nc.vector.tensor_add(dstn, psn, b_bcast4[:])
                for t in range(8):
                    nc.scalar.activation(
                        sqscr[:], projnat[:, t * P:(t + 1) * P].bitcast(F32),
                        AF.Square, accum_out=ksq[:, t:t + 1])
            # ---- main loop: j-tile-major, one ACT pass via custom table ----
            with (
                tc.tile_pool(name="rings", bufs=1) as rings,
                tc.tile_pool(name="psqk", bufs=3, space="PSUM") as psqk,
                tc.tile_pool(name="psk", bufs=1, space="PSUM") as psk,
            ):
                power_ring = rings.tile([P, RING], F32, name="power_ring")
                sim_ring = rings.tile([P, RING], F32R, name="sim_ring")
                kacc = [psk.tile([P, NS], F32, tag=f"kacc{q}",
                                 name=f"kacc{q}")
                        for q in range(2)]
                NRS = RING // NA   # 16 ring slots of 1024

                def emit_span_g2_affine(st, p):
                    # span p of stage st: j-tiles 4p..4p+3, i-half st
                    for jt in range(p * SPAN, (p + 1) * SPAN):
                        u = jt
                        ps = psqk.tile([P, NA], F32, tag="qk")
                        for g in range(2):
                            nc.tensor.matmul(
                                ps[:, g * NS:(g + 1) * NS],
                                projTm2[:, jt * P:(jt + 1) * P],
                                qT_mm[:, st * NA + g * NS:
                                      st * NA + (g + 1) * NS],
                                start=True, stop=True,
                            )
                        dst = power_ring[:, (u % NRS) * NA:
                                         (u % NRS) * NA + NA]
                        nc.vector.scalar_tensor_tensor(
                            dst, ps, ksq[:, jt:jt + 1],
                            qsq_bcast[:, st * NA:(st + 1) * NA],
                            op0=ALU.add, op1=ALU.add,
                        )

                B_SPANS = set()

                def emit_span_B(st, p):
                    # rank-1 trick: psum gets -2qk + qsq via PE; the g-table
                    # ACT op reads PSUM directly with bias=ksq (no DVE).
                    for jt in range(p * SPAN, (p + 1) * SPAN):
                        u = jt
                        ps = psqk.tile([P, NA], F32, tag="qk")
                        for g in range(2):
                            nc.tensor.matmul(
                                ps[:, g * NS:(g + 1) * NS],
                                projTm2[:, jt * P:(jt + 1) * P],
                                qT_mm[:, st * NA + g * NS:
                                      st * NA + (g + 1) * NS],
                                start=True, stop=False,
                            )

                        for g in range(2):
                            nc.tensor.matmul(
                                ps[:, g * NS:(g + 1) * NS], e0_sb[:],
                                qsq_bcast[:, st * NA + g * NS:
                                          st * NA + (g + 1) * NS].bitcast(F32R),
                                start=False, stop=True,
                            )
                        nc.scalar.activation(
                            sim_ring[:, (u % NRS) * NA:(u % NRS) * NA + NA],
                            ps, AF.Sqrt, bias=ksq[:, jt:jt + 1])

                def emit_span_g3_only(st, p):
                    for jt in range(p * SPAN, (p + 1) * SPAN):
                        u = jt
                        for g in range(2):
                            nc.tensor.matmul(
                                kacc[g],
                                projnat[:, jt * P:(jt + 1) * P],
                                sim_ring[:, (u % NRS) * NA + g * NS:
                                         (u % NRS) * NA + (g + 1) * NS],
                                start=(jt == 0), stop=(jt == NJT - 1),
                            )

                def emit_span_act_g3(st, p):
                    off = (p % (NRS // SPAN)) * SPAN * NA
                    nc.scalar.activation(
                        sim_ring[:, off:off + SPAN * NA],
                        power_ring[:, off:off + SPAN * NA], AF.Sqrt)
                    for jt in range(p * SPAN, (p + 1) * SPAN):
                        u = jt
                        for g in range(2):
                            nc.tensor.matmul(
                                kacc[g],
                                projnat[:, jt * P:(jt + 1) * P],
                                sim_ring[:, (u % NRS) * NA + g * NS:
                                         (u % NRS) * NA + (g + 1) * NS],
                                start=(jt == 0), stop=(jt == NJT - 1),
                            )

                NSP = NJT // SPAN   # spans per stage (8)
                sq_next = [8]

                def emit_deferred_squares(n):
                    while sq_next[0] < min(n, NJT):
                        t = sq_next[0]
                        nc.scalar.activation(
                            sqscr[:], projnat[:, t * P:(t + 1) * P].bitcast(F32),
                            AF.Square, accum_out=ksq[:, t:t + 1])
                        sq_next[0] += 1

                def emit_front(st, p):
                    if (st, p) in B_SPANS:
                        emit_span_B(st, p)
                    else:
                        emit_span_g2_affine(st, p)

                def emit_back(st, p):
                    if (st, p) in B_SPANS:
                        emit_span_g3_only(st, p)
                    else:
                        emit_span_act_g3(st, p)

                for st in range(2):
                    emit_front(st, 0)
                    emit_front(st, 1)
                    for p in range(NSP):
                        if st == 0:
                            emit_deferred_squares(SPAN * (p + 3) + 2)
                        if p + 2 < NSP:
                            emit_front(st, p + 2)
                        emit_back(st, p)
                    for g in range(2):
                        q = st * 2 + g
                        kT_tile = work.tile([P, NS], F32, tag="kout")
                        nc.scalar.copy(kT_tile[:], kacc[g])
                        nc.sync.dma_start(
                            out=kT_out[:, q * NS:(q + 1) * NS],
                            in_=kT_tile[:])

    nc.compile()
    return nc


_NC_CACHE = {}


def _get_nc():
    if "nc" not in _NC_CACHE:
        _NC_CACHE["nc"] = build_nc()
    return _NC_CACHE["nc"]


def make_in_maps(x, W, b):
    x = np.asarray(x, dtype=np.float32)
    W = np.asarray(W, dtype=np.float32)
    b = np.asarray(b, dtype=np.float32)

    WT = np.ascontiguousarray(W.T)
    WhT = np.ascontiguousarray((0.5 * W).T)
    bcol_h = np.ascontiguousarray((0.5 * b).reshape(P, 1).astype(np.float32))
    b_bcast4 = np.ascontiguousarray(
        np.tile(np.broadcast_to(b.reshape(1, D), (P, D)), (1, 4))
        .astype(np.float32))
    ones_mat = np.ones((P, P), np.float32)
    e0_mat = np.zeros((P, P), np.float32)
    e0_mat[0, :] = 1.0

    in_maps = []
    for i in range(B):
        in_maps.append({
            "xT": np.ascontiguousarray(x[i].T),
            "WT": WT,
            "WhT": WhT,
            "bcol_h": bcol_h,
            "b_bcast4": b_bcast4,
            "ones_mat": ones_mat,
            "e0_mat": e0_mat,
        })
    return in_maps


def kernel(x, W, b):
    nc = _get_nc()
    in_maps = make_in_maps(x, W, b)
    B = len(in_maps)

    trace = bool(int(os.environ.get("KBENCH_TRACE", "0")))
    kres = None
    last_exc = None
    for attempt in range(5):
        try:
            kres = run_bass_kernel_spmd(nc, in_maps, list(range(B)), trace=trace)
            break
        except Exception as exc:
            last_exc = exc
            import time as _time
            _time.sleep(3.0 * (attempt + 1))
    if kres is None:
        raise last_exc
    _NC_CACHE["last_result"] = kres
    res = kres.results

    q = np.stack([np.ascontiguousarray(r["qT"].T) for r in res])
    k = np.stack([np.ascontiguousarray(r["kT"].T) for r in res])
    return q, k, k

